# revision 36
# baseline (speedup 1.0000x reference)
"""Trainium2 Bass kernel for nn_FFT3D: AFNO-style 3D spectral block.

out = irfftn( softshrink( complexMLP( rfftn(x, axes=(D,H,W), ortho) ) ), ortho ) + x

Sharding: 8 cores, core n owns channel block n (32 channels) for both batches
(the channel MLP is block-diagonal with 8 blocks of 32 -> zero communication).

Per-core pipeline (validated in numpy first; see repo history):
  S1 rfft-W (matmul, DFT stationary) -> Y1 DRAM bounce
  S2 fft-H  (xbar transposed read puts (ri,h) on lanes; C2 complex matmul)
  PE-transpose -> S3 fft-D (real-form matmuls, c-oct groups)
  MLP1 + relu, MLP2 + softshrink (real-form, chi K-split, co-oct M-passes)
  S5 ifft-D, PE-transpose -> S6 ifft-H (C2), PE-transpose -> S7 irfft-W
  -> eps DRAM; host adds residual x in fp32.

All storage bf16, PSUM accumulation fp32. Expected rel err ~1e-4 (output is
x + small spectral correction; bf16 rounding enters only the correction).
"""
import numpy as np
import ml_dtypes

BF16NP = ml_dtypes.bfloat16

D, H, W, C = 16, 64, 64, 32
KW = 33
LAM = 0.01
N_CORES = 8
KW_CHUNKS = [(0, 8), (8, 8), (16, 8), (24, 8), (32, 1)]

_PROGRAM_CACHE = {}


# ------------------------------------------------------------------
# host-side stationary matrices (fp32 -> bf16)
# ------------------------------------------------------------------

def _host_stationaries():
    st = {}
    Fw = np.fft.rfft(np.eye(W), axis=0, norm='ortho')
    s1 = np.zeros((W, 66), np.float32)
    s1[:, 0:33] = Fw.real.T          # col s = ri*33 + kw
    s1[:, 33:66] = Fw.imag.T
    # 2-group block-diag: rows (g2, w64), cols (ri2, g2, kw33). One matmul
    # handles a chunk-PAIR (c-halves g=0/1 stacked on partitions) -> half the
    # LDW count; ri-major cols keep the drain AP at 3 free dims.
    s1b = np.zeros((128, 132), np.float32)
    for g in range(2):
        for r in range(2):
            s1b[g*64:(g+1)*64, r*66 + g*33: r*66 + g*33 + 33] = \
                s1[:, r*33:(r+1)*33]
    st['st1'] = s1b

    Fh = np.fft.fft(np.eye(H), axis=0, norm='ortho')
    s2A = np.zeros((64, 128), np.float32)       # psum = A.T@Yre + B.T@Yim -> [khRe|khIm]
    s2A[:, 0:64] = Fh.real.T
    s2A[:, 64:128] = Fh.imag.T
    s2B = np.zeros((64, 128), np.float32)
    s2B[:, 0:64] = -Fh.imag.T
    s2B[:, 64:128] = Fh.real.T
    # duplicated on both partition halves (matmul needs lhsT/rhs base match)
    st['st2a'] = np.concatenate([s2A, s2A], axis=0)
    st['st2b'] = np.concatenate([s2B, s2B], axis=0)

    Fd = np.fft.fft(np.eye(D), axis=0, norm='ortho')
    s3 = np.zeros((128, 3, 128), np.float32)   # [lanes, {r,i,-i}, m]
    for cl in range(8):
        blkr = Fd.real.T       # [d, kd]
        blki = Fd.imag.T
        s3[cl*16:(cl+1)*16, 0, cl*16:(cl+1)*16] = blkr
        s3[cl*16:(cl+1)*16, 1, cl*16:(cl+1)*16] = blki
        s3[cl*16:(cl+1)*16, 2, cl*16:(cl+1)*16] = -blki
    st['st3'] = s3

    Gd = np.conj(Fd).T         # [d, kd] inverse
    s5 = np.zeros((128, 3, 128), np.float32)
    for cl in range(8):
        s5[cl*16:(cl+1)*16, 0, cl*16:(cl+1)*16] = Gd.real      # lhsT[kd, d] = Gd[d? ] careful below
        s5[cl*16:(cl+1)*16, 1, cl*16:(cl+1)*16] = Gd.imag
        s5[cl*16:(cl+1)*16, 2, cl*16:(cl+1)*16] = -Gd.imag
    # lhsT[l=(cl,kd), m=(cl,d)] must equal Gd[d, kd] -> block = Gd.T
    s5[:, 0, :] = 0; s5[:, 1, :] = 0; s5[:, 2, :] = 0
    for cl in range(8):
        s5[cl*16:(cl+1)*16, 0, cl*16:(cl+1)*16] = Gd.real.T
        s5[cl*16:(cl+1)*16, 1, cl*16:(cl+1)*16] = Gd.imag.T
        s5[cl*16:(cl+1)*16, 2, cl*16:(cl+1)*16] = -Gd.imag.T
    st['st5'] = s5

    Gh = np.conj(Fh).T         # [h, kh]
    s6 = np.zeros((128, 128), np.float32)
    s6[0:64, 0:64] = Gh.real.T
    s6[64:128, 0:64] = -Gh.imag.T
    s6[0:64, 64:128] = Gh.imag.T
    s6[64:128, 64:128] = Gh.real.T
    st['st6'] = s6

    Cr = np.zeros((KW, W), np.float32)
    Ci = np.zeros((KW, W), np.float32)
    for k_ in range(KW):
        e = np.zeros(KW, np.complex64); e[k_] = 1.0
        Cr[k_] = np.fft.irfft(e, n=W, norm='ortho')
        e = np.zeros(KW, np.complex64); e[k_] = 1.0j
        Ci[k_] = np.fft.irfft(e, n=W, norm='ortho')
    s7 = np.zeros((66, 2, 128), np.float32)
    for kw_ in range(KW):
        for dlo in range(2):
            s7[kw_*2 + dlo, 0, dlo*64:(dlo+1)*64] = Cr[kw_]
            s7[kw_*2 + dlo, 1, dlo*64:(dlo+1)*64] = Ci[kw_]
    st['st7'] = s7

    st['ident'] = np.eye(128, dtype=np.float32)
    return st


def _host_mlp_stationaries(w1, w2, n):
    out = {}
    for name, wfull in (('stw1', w1), ('stw2', w2)):
        wr = np.asarray(wfull[0, n], np.float32)   # [32in, 32out]
        wi = np.asarray(wfull[1, n], np.float32)
        arr = np.zeros((128, 3, 4, 4, 128), np.float32)   # [lanes, which, q, chi, m]
        for which, wmat in ((0, wr), (1, wi), (2, -wi)):
            for q in range(4):
                for chi in range(4):
                    for cl in range(8):
                        for col in range(8):
                            v = wmat[chi*8 + cl, q*8 + col]
                            idx = np.arange(D)
                            arr[cl*16 + idx, which, q, chi, col*16 + idx] = v
        out[name] = arr
    return out


def _host_biases(b1, b2, n):
    b1n = np.asarray(b1[:, n], np.float32)   # [2, 32]
    b2n = np.asarray(b2[:, n], np.float32)
    p = np.arange(128)
    out = {}
    bias1 = np.zeros((128, 8), np.float32)
    bias2m = np.zeros((128, 8), np.float32)
    bias2p = np.zeros((128, 8), np.float32)
    bias2c = np.zeros((128, 8), np.float32)
    for ri in range(2):
        for q in range(4):
            co = q*8 + p//16
            bias1[:, ri*4 + q] = b1n[ri][co]
            bias2m[:, ri*4 + q] = b2n[ri][co] - LAM
            bias2p[:, ri*4 + q] = -b2n[ri][co] - LAM
            bias2c[:, ri*4 + q] = b2n[ri][co] + LAM
    out['bias1'], out['bias2m'], out['bias2p'] = bias1, bias2m, bias2p
    out['bias2c'] = bias2c
    return out


# ------------------------------------------------------------------
# the Bass program (identical for all cores; per-core data differs)
# ------------------------------------------------------------------

def _build_program():
    import concourse.bass as bass
    import concourse.bacc as bacc
    import concourse.mybir as mybir
    from concourse.tile import TileContext

    dt = mybir.dt
    BF = dt.bfloat16
    F32 = dt.float32
    FE4 = dt.float8e4
    Relu = mybir.ActivationFunctionType.Relu
    DR = mybir.MatmulPerfMode.DoubleRow

    nc = bacc.Bacc("TRN2", target_bir_lowering=False, debug=False)
    xin = nc.declare_dram_parameter("xin", [2, D, W, C, H], BF, isOutput=False)
    st1 = nc.declare_dram_parameter("st1", [128, 132], BF, isOutput=False)
    st2a = nc.declare_dram_parameter("st2a", [128, 128], BF, isOutput=False)
    st2b = nc.declare_dram_parameter("st2b", [128, 128], BF, isOutput=False)
    st3 = nc.declare_dram_parameter("st3", [128, 3, 128], BF, isOutput=False)
    st5 = nc.declare_dram_parameter("st5", [128, 3, 128], BF, isOutput=False)
    st6 = nc.declare_dram_parameter("st6", [128, 128], BF, isOutput=False)
    st7 = nc.declare_dram_parameter("st7", [66, 2, 128], BF, isOutput=False)
    stw1 = nc.declare_dram_parameter("stw1", [128, 3, 4, 4, 128], FE4, isOutput=False)
    stw2 = nc.declare_dram_parameter("stw2", [128, 3, 4, 4, 128], FE4, isOutput=False)
    bias1 = nc.declare_dram_parameter("bias1", [128, 8], F32, isOutput=False)
    bias2m = nc.declare_dram_parameter("bias2m", [128, 8], F32, isOutput=False)
    bias2p = nc.declare_dram_parameter("bias2p", [128, 8], F32, isOutput=False)
    bias2c = nc.declare_dram_parameter("bias2c", [128, 8], F32, isOutput=False)
    ident = nc.declare_dram_parameter("ident", [128, 128], BF, isOutput=False)
    eps = nc.declare_dram_parameter("eps", [2, 2, W, 4, 8, 8, H], BF, isOutput=True)
    warmout = nc.declare_dram_parameter("warmout", [128, 4], F32, isOutput=True)

    drain_ctr = [0]

    def drain(out_ap, in_ap, eng=None):
        # ACT is the busier engine (relu/softshrink live there) -> route
        # 3 of 4 copies to DVE, 1 of 4 to ACT. eng=0/1 forces scalar/vector.
        if eng is None:
            eng = 0 if (drain_ctr[0] % 4 == 3) else 1
            drain_ctr[0] += 1
        if eng == 0:
            nc.scalar.copy(out_ap, in_ap)
        else:
            nc.vector.tensor_copy(out_ap, in_ap)

    with TileContext(nc) as tc:
        with (
            tc.tile_pool(name="const", bufs=1) as cp,
            tc.tile_pool(name="spec", bufs=2) as specp,
            tc.tile_pool(name="specf", bufs=1) as specfp,
            tc.tile_pool(name="work", bufs=3) as wp,
            tc.tile_pool(name="xtb", bufs=2) as xtbp,
            tc.tile_pool(name="meb", bufs=2) as mebp,
            tc.tile_pool(name="outb", bufs=1) as outbp,
            tc.tile_pool(name="psA", bufs=2, space="PSUM") as psA,
            tc.tile_pool(name="psB", bufs=2, space="PSUM") as psB,
            tc.tile_pool(name="psT", bufs=2, space="PSUM") as psT,
            tc.tile_pool(name="psM", bufs=2, space="PSUM") as psM,
        ):
            # ---- load constants ----
            def cload(shape, dtype, src):
                t = cp.tile(shape, dtype, tag=src.name)
                nc.sync.dma_start(out=t[:], in_=src[:])
                return t
            t_id = cload([128, 128], BF, ident)
            # PE warm-up: ~4.5us of back-to-back matmuls so the HAM clock
            # gate is at 8/8 before S1's first real matmul (runs during
            # the xin DMA wait).
            wm = psA.tile([128, 128], F32, tag="a")
            for _ in range(40):
                nc.tensor.matmul(wm[:], t_id[:], t_id[:], start=True, stop=True)
            wt = wp.tile([128, 4], F32, tag="wmv")
            nc.vector.tensor_copy(wt[:], wm[:, 0:4])
            nc.sync.dma_start(out=warmout[:], in_=wt[:])
            # order: small/early-stage consts first; the 3MB MLP weights
            # last so they don't delay the first xtb input tile.
            t_s1 = cload([128, 132], BF, st1)
            t_s2a = cload([128, 128], BF, st2a)
            t_s2b = cload([128, 128], BF, st2b)
            t_s3 = cload([128, 3, 128], BF, st3)
            t_b1 = cload([128, 8], F32, bias1)
            t_b2m = cload([128, 8], F32, bias2m)
            t_b2p = cload([128, 8], F32, bias2p)
            t_b2c = cload([128, 8], F32, bias2c)
            t_s5 = cload([128, 3, 128], BF, st5)
            t_s6 = cload([128, 128], BF, st6)
            t_s7 = cload([66, 2, 128], BF, st7)
            t_w1 = cload([128, 3, 4, 4, 128], FE4, stw1)
            t_w2 = cload([128, 3, 4, 4, 128], FE4, stw2)

            # ---- per-batch stage emitters; called in a custom order so that
            # batch-0's drain-bound S7 interleaves with batch-1's S1/S2 ----
            st_ = {}

            def alloc_spec(b):
                specB = specp.tile([128, D*KW*C], BF, tag="spec")
                # layout: F = (chi4, cl8, d16, kw33); c = (cph, cpv, cpl, par),
                # chi = (cph, cpv), cl = (cpl, par) -> chi-octet is one
                # single-stride 128-slice for the S3 transposes.
                st_[b, 'specBv'] = specB[:].rearrange(
                    "p (cph cpv cpl par d k) -> p cph cpv cpl par d k",
                    cph=2, cpv=2, cpl=4, par=2, d=D)
                st_[b, 'specBc'] = specB[:].rearrange("p (a k) -> p a k", k=KW)

            def emit_dh_dma(b, dh):
                # rows = (g2, w64) where g = c-half (c = g*16 + cl); HBM runs
                # stay 2KB (c16 x h64 contiguous per (g, w, d)).
                xtb = xtbp.tile([128, 4*(C//2)*H], BF, tag="xtb")
                for g in range(2):
                    nc.scalar.dma_start(
                        out=xtb[g*64:(g+1)*64].rearrange(
                            "p (d c h) -> p d c h", d=4, c=C//2),
                        in_=xin[b, dh*4:(dh+1)*4, :, g*16:(g+1)*16].rearrange(
                            "d w c h -> w d c h"))
                st_[b, 'xtb'] = xtb

            def emit_s1s2_dd(b, d_):
                dd = d_ % 4
                xtb = st_[b, 'xtb']
                specBv = st_[b, 'specBv']
                # S1 data-stationary chunk-PAIR: lhsT [128=(g2,w64), 128 =
                # (c2,h64)], rhs block-diag DFT [128, 132=(g2,kwri66)].
                # psum [(c2,h64), (g2,ri2,kw33)] -> movB (ri2, g2, t8, kw33)
                # so the S2 rhs slices are fully CONTIGUOUS [64, 264].
                movB = wp.tile([128, 16*66], BF, tag="movB")
                movBv = movB[:].rearrange("p (rg t k) -> p rg t k", rg=4, t=8)
                for j0 in range(0, 8, 2):
                    ps = psM.tile([128, 264], F32, tag="m")
                    for jj in range(2):
                        j = j0 + jj
                        nc.tensor.matmul(ps[:, jj*132:(jj+1)*132],
                                         xtb[:, dd*1024 + j*128: dd*1024 + (j+1)*128],
                                         t_s1[:], start=True, stop=True)
                    # psum chunk cols (ri2, g2, kw33); movB cols (ri2, g2, t8, kw33)
                    drain(movBv[:, :, j0:j0+2, :],
                          ps[:].rearrange("p (j rg k) -> p rg j k", j=2, rg=4))
                # S2: par = c parity (lane half), cph = c-half octet.
                # All A-matmuls first, then all B (4 LDW/dd instead of 8);
                # 4 psum banks live (2 from psA + 2 from psB).
                pss = []
                for par in range(2):
                    sl = slice(par*64, (par+1)*64)
                    for cph in range(2):
                        mre = movB[sl, cph*264: (cph+1)*264]
                        mim = movB[sl, 528 + cph*264: 528 + (cph+1)*264]
                        pool = psA if cph == 0 else psB
                        ps = pool.tile([128, 8*KW], F32,
                                       tag="a" if cph == 0 else "b")
                        pss.append((ps, sl, mre, mim))
                for (ps, sl, mre, mim) in pss:
                    nc.tensor.matmul(ps[:], t_s2a[sl, :], mre, start=True, stop=False)
                for (ps, sl, mre, mim) in pss:
                    nc.tensor.matmul(ps[:], t_s2b[sl, :], mim, start=False, stop=True)
                for i, (ps, sl, mre, mim) in enumerate(pss):
                    par, cph = i // 2, i % 2
                    # psum cols (cp8, kw33) -> specB[(chi,cl) c-order]
                    drain(specBv[:, cph, :, :, par, d_, :],
                          ps[:].rearrange("p (v l k) -> p v l k", v=2, l=4))

            def emit_mid(b):
                specBc = st_[b, 'specBc']
                # ============ S2->S3 transpose + S3: fft over D ============
                # y3/y4 are fp8e4: the MLP runs fp8 DoubleRow matmuls (2
                # chi-planes contracted per instruction). Spectral values are
                # O(1) corrections; fp8's ~3% rounding stays well inside the
                # error budget.
                y3 = specp.tile([128, KW*4*2*64], FE4, tag="spec")
                y3v = y3[:].rearrange("p (c r k x) -> p c r k x", c=4, r=2, k=KW)
                y3f = y3[:].rearrange("p (c r kx) -> p c r kx", c=4, r=2)
                for kw_ in range(KW):
                    movT = wp.tile([128, 512], BF, tag="movT")
                    # 4 chi-transposes into one psum bank -> single drain
                    pt = psT.tile([128, 512], BF, tag="t")
                    for chi in range(4):
                        nc.tensor.transpose(
                            pt[:, chi*128:(chi+1)*128],
                            specBc[:, chi*128:(chi+1)*128, kw_], t_id[:])
                    drain(movT[:].rearrange("p (r c x) -> p r c x", r=2, c=4),
                          pt[:].rearrange("p (c r x) -> p r c x", c=4, r=2))
                    mov_re = movT[:, 0:256]
                    mov_im = movT[:, 256:512]
                    pr = psA.tile([128, 256], F32, tag="a")
                    pi = psB.tile([128, 256], F32, tag="b")
                    # s3[0] stationary reused by consecutive matmuls (1 LDW saved)
                    nc.tensor.matmul(pr[:], t_s3[:, 0, :], mov_re, start=True, stop=False)
                    nc.tensor.matmul(pi[:], t_s3[:, 0, :], mov_im, start=True, stop=False)
                    nc.tensor.matmul(pr[:], t_s3[:, 2, :], mov_im, start=False, stop=True)
                    nc.tensor.matmul(pi[:], t_s3[:, 1, :], mov_re, start=False, stop=True)
                    drain(y3v[:, :, 0, kw_, :], pr[:].rearrange("p (c x) -> p c x", c=4))
                    drain(y3v[:, :, 1, kw_, :], pi[:].rearrange("p (c x) -> p c x", c=4))

                # ================= MLP layer 1 (+bias+relu) =================
                y4 = specp.tile([128, KW*4*2*64], FE4, tag="spec")
                y4v = y4[:].rearrange("p (c r k x) -> p c r k x", c=4, r=2, k=KW)
                y4f = y4[:].rearrange("p (c r kx) -> p c r kx", c=4, r=2)
                for q in range(4):
                    for (k0, kn) in KW_CHUNKS:
                        if kn == 1:
                            # psM/psT are idle during the MLP: extra PSUM
                            # rotation depth at the q-pass boundary
                            pr = psM.tile([128, kn*64], F32, tag="m")
                            pi = psT.tile([128, kn*64], F32, tag="t")
                        else:
                            pr = psA.tile([128, kn*64], F32, tag="a")
                            pi = psB.tile([128, kn*64], F32, tag="b")
                        for cp_ in range(2):
                            rre = y3f[:, 2*cp_:2*cp_+2, 0, k0*64:(k0+kn)*64]
                            rim = y3f[:, 2*cp_:2*cp_+2, 1, k0*64:(k0+kn)*64]
                            nc.tensor.matmul(pr[:], t_w1[:, 0, q, 2*cp_:2*cp_+2, :], rre,
                                             start=(cp_ == 0), stop=False, perf_mode=DR)
                            nc.tensor.matmul(pi[:], t_w1[:, 0, q, 2*cp_:2*cp_+2, :], rim,
                                             start=(cp_ == 0), stop=False, perf_mode=DR)
                            nc.tensor.matmul(pr[:], t_w1[:, 2, q, 2*cp_:2*cp_+2, :], rim,
                                             start=False, stop=(cp_ == 1), perf_mode=DR)
                            nc.tensor.matmul(pi[:], t_w1[:, 1, q, 2*cp_:2*cp_+2, :], rre,
                                             start=False, stop=(cp_ == 1), perf_mode=DR)
                        nc.scalar.activation(
                            y4v[:, q, 0, k0:k0+kn, :],
                            pr[:].rearrange("p (k x) -> p k x", k=kn),
                            Relu, bias=t_b1[:, q:q+1], scale=1.0)
                        nc.scalar.activation(
                            y4v[:, q, 1, k0:k0+kn, :],
                            pi[:].rearrange("p (k x) -> p k x", k=kn),
                            Relu, bias=t_b1[:, 4+q:4+q+1], scale=1.0)

                # ================= MLP layer 2 (+bias+softshrink) =================
                y5 = specp.tile([128, KW*4*2*64], BF, tag="spec")
                y5v = y5[:].rearrange("p (c r k x) -> p c r k x", c=4, r=2, k=KW)
                for q in range(4):
                    for (k0, kn) in KW_CHUNKS:
                        if kn == 1:
                            # psM/psT are idle during the MLP: extra PSUM
                            # rotation depth at the q-pass boundary
                            pr = psM.tile([128, kn*64], F32, tag="m")
                            pi = psT.tile([128, kn*64], F32, tag="t")
                        else:
                            pr = psA.tile([128, kn*64], F32, tag="a")
                            pi = psB.tile([128, kn*64], F32, tag="b")
                        for cp_ in range(2):
                            rre = y4f[:, 2*cp_:2*cp_+2, 0, k0*64:(k0+kn)*64]
                            rim = y4f[:, 2*cp_:2*cp_+2, 1, k0*64:(k0+kn)*64]
                            nc.tensor.matmul(pr[:], t_w2[:, 0, q, 2*cp_:2*cp_+2, :], rre,
                                             start=(cp_ == 0), stop=False, perf_mode=DR)
                            nc.tensor.matmul(pi[:], t_w2[:, 0, q, 2*cp_:2*cp_+2, :], rim,
                                             start=(cp_ == 0), stop=False, perf_mode=DR)
                            nc.tensor.matmul(pr[:], t_w2[:, 2, q, 2*cp_:2*cp_+2, :], rim,
                                             start=False, stop=(cp_ == 1), perf_mode=DR)
                            nc.tensor.matmul(pi[:], t_w2[:, 1, q, 2*cp_:2*cp_+2, :], rre,
                                             start=False, stop=(cp_ == 1), perf_mode=DR)
                        for (ps_, ri) in ((pr, 0), (pi, 1)):
                            ta = wp.tile([128, 512], BF, tag="ssA")
                            tb = wp.tile([128, 512], BF, tag="ssB")
                            # softshrink spread over 3 engines:
                            # ss(v) = max(v-lam, 0) + min(v+lam, 0), v = ps+b2
                            # ta = relu branch on ACT, tb = min branch on DVE,
                            # final add on the (otherwise idle) GPSIMD.
                            nc.scalar.activation(ta[:, :kn*64], ps_[:], Relu,
                                                 bias=t_b2m[:, ri*4+q:ri*4+q+1], scale=1.0)
                            nc.vector.tensor_scalar(tb[:, :kn*64], ps_[:],
                                                    t_b2c[:, ri*4+q:ri*4+q+1], 0.0,
                                                    mybir.AluOpType.add,
                                                    mybir.AluOpType.min)
                            nc.gpsimd.tensor_add(
                                y5v[:, q, ri, k0:k0+kn, :],
                                ta[:, :kn*64].rearrange("p (k x) -> p k x", k=kn),
                                tb[:, :kn*64].rearrange("p (k x) -> p k x", k=kn))

                # ================= S5: ifft over D =================
                specE = specp.tile([128, 4*KW*2*64], BF, tag="spec")
                specEv = specE[:].rearrange("p (q k r x) -> p q k r x", q=4, k=KW, r=2)
                for q in range(4):
                    for (k0, kn) in KW_CHUNKS:
                        pr = psA.tile([128, kn*64], F32, tag="a")
                        pi = psB.tile([128, kn*64], F32, tag="b")
                        rre = y5v[:, q, 0, k0:k0+kn, :]
                        rim = y5v[:, q, 1, k0:k0+kn, :]
                        nc.tensor.matmul(pr[:], t_s5[:, 0, :], rre, start=True, stop=False)
                        nc.tensor.matmul(pi[:], t_s5[:, 0, :], rim, start=True, stop=False)
                        nc.tensor.matmul(pr[:], t_s5[:, 2, :], rim, start=False, stop=True)
                        nc.tensor.matmul(pi[:], t_s5[:, 1, :], rre, start=False, stop=True)
                        drain(specEv[:, q, k0:k0+kn, 0, :],
                              pr[:].rearrange("p (k x) -> p k x", k=kn))
                        drain(specEv[:, q, k0:k0+kn, 1, :],
                              pi[:].rearrange("p (k x) -> p k x", k=kn))

                st_[b, 'specE'] = specE

            def emit_s6_q(b, q):
                # ============ S5->S6 transpose + S6: ifft over H (C2) ============
                if q == 0:
                    specF = specp.tile([128, 4*8*8*KW*2], BF, tag="spec")
                    st_[b, 'specF'] = specF
                specE = st_[b, 'specE']
                specF = st_[b, 'specF']
                # Phase 1: all 33 transposes, 4 per psum bank -> batched
                # contiguous drains.
                meb = mebp.tile([128, KW*128], BF, tag="meb")
                for kw0 in range(0, KW, 4):
                    kn = min(4, KW - kw0)
                    pt = psT.tile([128, 512], BF, tag="t")
                    for kk in range(kn):
                        nc.tensor.transpose(
                            pt[:, kk*128:(kk+1)*128],
                            specE[:, (q*KW + kw0 + kk)*128:
                                     (q*KW + kw0 + kk + 1)*128],
                            t_id[:])
                    drain(meb[:, kw0*128:(kw0+kn)*128], pt[:, :kn*128])
                # Phase 2: matmuls share ONE persistent stationary (t_s6
                # stays loaded), 4 kw per psum bank -> batched drains.
                for kw0 in range(0, KW, 4):
                    kn = min(4, KW - kw0)
                    ps6 = psM.tile([128, 512], F32, tag="m")
                    for kk in range(kn):
                        nc.tensor.matmul(ps6[:, kk*128:(kk+1)*128], t_s6[:],
                                         meb[:, (kw0+kk)*128:(kw0+kk+1)*128],
                                         start=True, stop=True)
                    drain(specF[:].rearrange("p (q ce k l) -> p q ce k l",
                                             q=4, ce=64, k=KW)[:, q, :, kw0:kw0+kn, :],
                          ps6[:, :kn*128].rearrange(
                              "p (kk ce l) -> p ce kk l", kk=kn, ce=64))

            def emit_s7_group(b, gi):
                # ============ S6->S7 transpose + S7: irfft over W ============
                if gi == 0:
                    outbuf = outbp.tile([128, 4*8*8*H], BF, tag="outbuf")
                    st_[b, 'outbuf'] = outbuf
                specF = st_[b, 'specF']
                outbuf = st_[b, 'outbuf']
                q, cl = gi // 8, gi % 8
                movF = wp.tile([66, 8*128], BF, tag="movF")
                for e0 in range(0, 8, 4):
                    # 4 transposes per psum bank -> one contiguous drain
                    pt = psT.tile([128, 512], BF, tag="t")
                    for ee in range(4):
                        base = (gi*8 + e0 + ee)*66
                        nc.tensor.transpose(pt[:66, ee*128:(ee+1)*128],
                                            specF[:, base:base+66], t_id[:])
                    drain(movF[:, e0*128:(e0+4)*128], pt[:66, :])
                movFv = movF[:].rearrange("p (e g) -> p e g", e=8)
                ps7 = psM.tile([128, 512], F32, tag="m")
                # alternate stationary order by group parity so adjacent
                # groups share a loaded stationary (halves LDW count)
                if gi % 2 == 0:
                    nc.tensor.matmul(ps7[:], t_s7[:, 0, :], movFv[:, :, 0:64],
                                     start=True, stop=False)
                    nc.tensor.matmul(ps7[:], t_s7[:, 1, :], movFv[:, :, 64:128],
                                     start=False, stop=True)
                else:
                    nc.tensor.matmul(ps7[:], t_s7[:, 1, :], movFv[:, :, 64:128],
                                     start=True, stop=False)
                    nc.tensor.matmul(ps7[:], t_s7[:, 0, :], movFv[:, :, 0:64],
                                     start=False, stop=True)
                # all outbuf drains on one engine so the eps DMA
                # waits on a single producer sem
                drain(outbuf[:, gi*8*H:(gi+1)*8*H], ps7[:], eng=0)
                if cl == 7:
                    # per-q quarter DMA as soon as its 8 cl-drains land
                    epsv = eps[b].rearrange("l w q c e h -> (l w) (q c e h)")
                    nc.sync.dma_start(
                        out=epsv[:, q*8*8*H:(q+1)*8*8*H],
                        in_=outbuf[:, q*8*8*H:(q+1)*8*8*H])

            # ---- emission schedule: batch 0 straight through to S6, then
            # batch-0 S7 groups interleaved with batch-1 S1/S2 (keeps the
            # in-order PE queue fed while S7's drains run on DVE/ACT), then
            # batch-1 mid stages and its S7 tail. ----
            alloc_spec(0)
            for dh in range(4):
                emit_dh_dma(0, dh)
                for dd in range(4):
                    emit_s1s2_dd(0, dh*4 + dd)
            emit_mid(0)
            for q in range(4):
                emit_s6_q(0, q)
            alloc_spec(1)
            for gi in range(32):
                emit_s7_group(0, gi)
                if gi % 2 == 0:
                    d_ = gi // 2
                    if d_ % 4 == 0:
                        emit_dh_dma(1, d_ // 4)
                    emit_s1s2_dd(1, d_)
            emit_mid(1)
            for q in range(4):
                emit_s6_q(1, q)
            for gi in range(32):
                emit_s7_group(1, gi)
    nc.compile()   # bacc passes: splits >1-wait instructions (HW limit)
    return nc


def _get_program():
    if 'nc' not in _PROGRAM_CACHE:
        _PROGRAM_CACHE['nc'] = _build_program()
    return _PROGRAM_CACHE['nc']


# ------------------------------------------------------------------
# host entry point
# ------------------------------------------------------------------

def make_core_inputs(x, w1, b1, w2, b2, n):
    """Build the per-core input map (numpy arrays) for core n."""
    st = _host_stationaries()
    xc = np.asarray(x[..., n*32:(n+1)*32], np.float32)       # [2, D, H, W, 32]
    xt = np.ascontiguousarray(xc.transpose(0, 1, 3, 4, 2))   # [2, D, W, C, H]
    m = {'xin': xt.astype(BF16NP)}
    for k in ('st1', 'st2a', 'st2b', 'st3', 'st5', 'st6', 'st7', 'ident'):
        m[k] = st[k].astype(BF16NP)
    mm = _host_mlp_stationaries(np.asarray(w1), np.asarray(w2), n)
    m['stw1'] = mm['stw1'].astype(ml_dtypes.float8_e4m3)
    m['stw2'] = mm['stw2'].astype(ml_dtypes.float8_e4m3)
    bb = _host_biases(np.asarray(b1), np.asarray(b2), n)
    m['bias1'] = bb['bias1']
    m['bias2m'] = bb['bias2m']
    m['bias2p'] = bb['bias2p']
    m['bias2c'] = bb['bias2c']
    return m


def eps_to_full(eps_n):
    """eps [2, dlo2, w64, q4, cl8, dhi8, h64] -> [2,D,H,W,32] fp32."""
    e = np.asarray(eps_n, dtype=np.float32)
    e = e.transpose(0, 5, 1, 6, 2, 3, 4)      # [b, dhi, dlo, h, w, q, cl]
    return np.ascontiguousarray(e.reshape(2, D, H, W, 32))


def kernel(x, w1, b1, w2, b2):
    from concourse.bass_utils import run_bass_kernel_spmd
    nc = _get_program()
    x = np.asarray(x)
    in_maps = [make_core_inputs(x, w1, b1, w2, b2, n) for n in range(N_CORES)]
    res = run_bass_kernel_spmd(nc, in_maps, list(range(N_CORES)))
    out = np.empty(x.shape, np.float32)
    for n in range(N_CORES):
        eps_full = eps_to_full(res.results[n]['eps'])
        out[..., n*32:(n+1)*32] = x[..., n*32:(n+1)*32].astype(np.float32) + eps_full
    return out



# revision 37
# speedup vs baseline: 1.0190x; 1.0190x over previous
"""Trainium2 Bass kernel for nn_FFT3D: AFNO-style 3D spectral block.

out = irfftn( softshrink( complexMLP( rfftn(x, axes=(D,H,W), ortho) ) ), ortho ) + x

Sharding: 8 cores, core n owns channel block n (32 channels) for both batches
(the channel MLP is block-diagonal with 8 blocks of 32 -> zero communication).

Per-core pipeline (validated in numpy first; see repo history):
  S1 rfft-W (matmul, DFT stationary) -> Y1 DRAM bounce
  S2 fft-H  (xbar transposed read puts (ri,h) on lanes; C2 complex matmul)
  PE-transpose -> S3 fft-D (real-form matmuls, c-oct groups)
  MLP1 + relu, MLP2 + softshrink (real-form, chi K-split, co-oct M-passes)
  S5 ifft-D, PE-transpose -> S6 ifft-H (C2), PE-transpose -> S7 irfft-W
  -> eps DRAM; host adds residual x in fp32.

All storage bf16, PSUM accumulation fp32. Expected rel err ~1e-4 (output is
x + small spectral correction; bf16 rounding enters only the correction).
"""
import numpy as np
import ml_dtypes

BF16NP = ml_dtypes.bfloat16

D, H, W, C = 16, 64, 64, 32
KW = 33
LAM = 0.01
N_CORES = 8
KW_CHUNKS = [(0, 8), (8, 8), (16, 8), (24, 8), (32, 1)]

_PROGRAM_CACHE = {}


# ------------------------------------------------------------------
# host-side stationary matrices (fp32 -> bf16)
# ------------------------------------------------------------------

def _host_stationaries():
    st = {}
    Fw = np.fft.rfft(np.eye(W), axis=0, norm='ortho')
    s1 = np.zeros((W, 66), np.float32)
    s1[:, 0:33] = Fw.real.T          # col s = ri*33 + kw
    s1[:, 33:66] = Fw.imag.T
    # 2-group block-diag: rows (g2, w64), cols (ri2, g2, kw33). One matmul
    # handles a chunk-PAIR (c-halves g=0/1 stacked on partitions) -> half the
    # LDW count; ri-major cols keep the drain AP at 3 free dims.
    s1b = np.zeros((128, 132), np.float32)
    for g in range(2):
        for r in range(2):
            s1b[g*64:(g+1)*64, r*66 + g*33: r*66 + g*33 + 33] = \
                s1[:, r*33:(r+1)*33]
    st['st1'] = s1b

    Fh = np.fft.fft(np.eye(H), axis=0, norm='ortho')
    s2A = np.zeros((64, 128), np.float32)       # psum = A.T@Yre + B.T@Yim -> [khRe|khIm]
    s2A[:, 0:64] = Fh.real.T
    s2A[:, 64:128] = Fh.imag.T
    s2B = np.zeros((64, 128), np.float32)
    s2B[:, 0:64] = -Fh.imag.T
    s2B[:, 64:128] = Fh.real.T
    # duplicated on both partition halves (matmul needs lhsT/rhs base match)
    st['st2a'] = np.concatenate([s2A, s2A], axis=0)
    st['st2b'] = np.concatenate([s2B, s2B], axis=0)

    Fd = np.fft.fft(np.eye(D), axis=0, norm='ortho')
    s3 = np.zeros((128, 3, 128), np.float32)   # [lanes, {r,i,-i}, m]
    for cl in range(8):
        blkr = Fd.real.T       # [d, kd]
        blki = Fd.imag.T
        s3[cl*16:(cl+1)*16, 0, cl*16:(cl+1)*16] = blkr
        s3[cl*16:(cl+1)*16, 1, cl*16:(cl+1)*16] = blki
        s3[cl*16:(cl+1)*16, 2, cl*16:(cl+1)*16] = -blki
    st['st3'] = s3

    Gd = np.conj(Fd).T         # [d, kd] inverse
    s5 = np.zeros((128, 3, 128), np.float32)
    for cl in range(8):
        s5[cl*16:(cl+1)*16, 0, cl*16:(cl+1)*16] = Gd.real      # lhsT[kd, d] = Gd[d? ] careful below
        s5[cl*16:(cl+1)*16, 1, cl*16:(cl+1)*16] = Gd.imag
        s5[cl*16:(cl+1)*16, 2, cl*16:(cl+1)*16] = -Gd.imag
    # lhsT[l=(cl,kd), m=(cl,d)] must equal Gd[d, kd] -> block = Gd.T
    s5[:, 0, :] = 0; s5[:, 1, :] = 0; s5[:, 2, :] = 0
    for cl in range(8):
        s5[cl*16:(cl+1)*16, 0, cl*16:(cl+1)*16] = Gd.real.T
        s5[cl*16:(cl+1)*16, 1, cl*16:(cl+1)*16] = Gd.imag.T
        s5[cl*16:(cl+1)*16, 2, cl*16:(cl+1)*16] = -Gd.imag.T
    st['st5'] = s5

    Gh = np.conj(Fh).T         # [h, kh]
    s6 = np.zeros((128, 128), np.float32)
    s6[0:64, 0:64] = Gh.real.T
    s6[64:128, 0:64] = -Gh.imag.T
    s6[0:64, 64:128] = Gh.imag.T
    s6[64:128, 64:128] = Gh.real.T
    st['st6'] = s6

    Cr = np.zeros((KW, W), np.float32)
    Ci = np.zeros((KW, W), np.float32)
    for k_ in range(KW):
        e = np.zeros(KW, np.complex64); e[k_] = 1.0
        Cr[k_] = np.fft.irfft(e, n=W, norm='ortho')
        e = np.zeros(KW, np.complex64); e[k_] = 1.0j
        Ci[k_] = np.fft.irfft(e, n=W, norm='ortho')
    s7 = np.zeros((66, 2, 128), np.float32)
    for kw_ in range(KW):
        for dlo in range(2):
            s7[kw_*2 + dlo, 0, dlo*64:(dlo+1)*64] = Cr[kw_]
            s7[kw_*2 + dlo, 1, dlo*64:(dlo+1)*64] = Ci[kw_]
    st['st7'] = s7

    st['ident'] = np.eye(128, dtype=np.float32)
    return st


def _host_mlp_stationaries(w1, w2, n):
    out = {}
    for name, wfull in (('stw1', w1), ('stw2', w2)):
        wr = np.asarray(wfull[0, n], np.float32)   # [32in, 32out]
        wi = np.asarray(wfull[1, n], np.float32)
        arr = np.zeros((128, 3, 4, 4, 128), np.float32)   # [lanes, which, q, chi, m]
        for which, wmat in ((0, wr), (1, wi), (2, -wi)):
            for q in range(4):
                for chi in range(4):
                    for cl in range(8):
                        for col in range(8):
                            v = wmat[chi*8 + cl, q*8 + col]
                            idx = np.arange(D)
                            arr[cl*16 + idx, which, q, chi, col*16 + idx] = v
        out[name] = arr
    return out


def _host_biases(b1, b2, n):
    b1n = np.asarray(b1[:, n], np.float32)   # [2, 32]
    b2n = np.asarray(b2[:, n], np.float32)
    p = np.arange(128)
    out = {}
    bias1 = np.zeros((128, 8), np.float32)
    bias2m = np.zeros((128, 8), np.float32)
    bias2p = np.zeros((128, 8), np.float32)
    bias2c = np.zeros((128, 8), np.float32)
    for ri in range(2):
        for q in range(4):
            co = q*8 + p//16
            bias1[:, ri*4 + q] = b1n[ri][co]
            bias2m[:, ri*4 + q] = b2n[ri][co] - LAM
            bias2p[:, ri*4 + q] = -b2n[ri][co] - LAM
            bias2c[:, ri*4 + q] = b2n[ri][co] + LAM
    out['bias1'], out['bias2m'], out['bias2p'] = bias1, bias2m, bias2p
    out['bias2c'] = bias2c
    return out


# ------------------------------------------------------------------
# the Bass program (identical for all cores; per-core data differs)
# ------------------------------------------------------------------

def _build_program():
    import concourse.bass as bass
    import concourse.bacc as bacc
    import concourse.mybir as mybir
    from concourse.tile import TileContext

    dt = mybir.dt
    BF = dt.bfloat16
    F32 = dt.float32
    FE4 = dt.float8e4
    Relu = mybir.ActivationFunctionType.Relu
    DR = mybir.MatmulPerfMode.DoubleRow

    nc = bacc.Bacc("TRN2", target_bir_lowering=False, debug=False)
    xin = nc.declare_dram_parameter("xin", [2, D, W, C, H], BF, isOutput=False)
    st1 = nc.declare_dram_parameter("st1", [128, 132], BF, isOutput=False)
    st2a = nc.declare_dram_parameter("st2a", [128, 128], BF, isOutput=False)
    st2b = nc.declare_dram_parameter("st2b", [128, 128], BF, isOutput=False)
    st3 = nc.declare_dram_parameter("st3", [128, 3, 128], BF, isOutput=False)
    st5 = nc.declare_dram_parameter("st5", [128, 3, 128], BF, isOutput=False)
    st6 = nc.declare_dram_parameter("st6", [128, 128], BF, isOutput=False)
    st7 = nc.declare_dram_parameter("st7", [66, 2, 128], BF, isOutput=False)
    stw1 = nc.declare_dram_parameter("stw1", [128, 3, 4, 4, 128], FE4, isOutput=False)
    stw2 = nc.declare_dram_parameter("stw2", [128, 3, 4, 4, 128], FE4, isOutput=False)
    bias1 = nc.declare_dram_parameter("bias1", [128, 8], F32, isOutput=False)
    bias2m = nc.declare_dram_parameter("bias2m", [128, 8], F32, isOutput=False)
    bias2p = nc.declare_dram_parameter("bias2p", [128, 8], F32, isOutput=False)
    bias2c = nc.declare_dram_parameter("bias2c", [128, 8], F32, isOutput=False)
    ident = nc.declare_dram_parameter("ident", [128, 128], BF, isOutput=False)
    eps = nc.declare_dram_parameter("eps", [2, 2, W, 4, 8, 8, H], BF, isOutput=True)
    warmout = nc.declare_dram_parameter("warmout", [128, 4], F32, isOutput=True)

    drain_ctr = [0]

    def drain(out_ap, in_ap, eng=None):
        # ACT is the busier engine (relu/softshrink live there) -> route
        # 3 of 4 copies to DVE, 1 of 4 to ACT. eng=0/1 forces scalar/vector.
        if eng is None:
            eng = 0 if (drain_ctr[0] % 4 == 3) else 1
            drain_ctr[0] += 1
        if eng == 0:
            nc.scalar.copy(out_ap, in_ap)
        else:
            nc.vector.tensor_copy(out_ap, in_ap)

    with TileContext(nc) as tc:
        with (
            tc.tile_pool(name="const", bufs=1) as cp,
            tc.tile_pool(name="spec", bufs=2) as specp,
            tc.tile_pool(name="specf", bufs=1) as specfp,
            tc.tile_pool(name="work", bufs=3) as wp,
            tc.tile_pool(name="xtb", bufs=2) as xtbp,
            tc.tile_pool(name="meb", bufs=2) as mebp,
            tc.tile_pool(name="outb", bufs=1) as outbp,
            tc.tile_pool(name="psA", bufs=2, space="PSUM") as psA,
            tc.tile_pool(name="psB", bufs=2, space="PSUM") as psB,
            tc.tile_pool(name="psT", bufs=2, space="PSUM") as psT,
            tc.tile_pool(name="psM", bufs=2, space="PSUM") as psM,
        ):
            # ---- load constants ----
            def cload(shape, dtype, src):
                t = cp.tile(shape, dtype, tag=src.name)
                nc.sync.dma_start(out=t[:], in_=src[:])
                return t
            t_id = cload([128, 128], BF, ident)
            # PE warm-up: ~4.5us of back-to-back matmuls so the HAM clock
            # gate is at 8/8 before S1's first real matmul (runs during
            # the xin DMA wait).
            wm = psA.tile([128, 128], F32, tag="a")
            for _ in range(40):
                nc.tensor.matmul(wm[:], t_id[:], t_id[:], start=True, stop=True)
            wt = wp.tile([128, 4], F32, tag="wmv")
            nc.vector.tensor_copy(wt[:], wm[:, 0:4])
            nc.sync.dma_start(out=warmout[:], in_=wt[:])
            # order: small/early-stage consts first; the 3MB MLP weights
            # last so they don't delay the first xtb input tile.
            t_s1 = cload([128, 132], BF, st1)
            t_s2a = cload([128, 128], BF, st2a)
            t_s2b = cload([128, 128], BF, st2b)
            t_s3 = cload([128, 3, 128], BF, st3)
            t_b1 = cload([128, 8], F32, bias1)
            t_b2m = cload([128, 8], F32, bias2m)
            t_b2p = cload([128, 8], F32, bias2p)
            t_b2c = cload([128, 8], F32, bias2c)
            t_s5 = cload([128, 3, 128], BF, st5)
            t_s6 = cload([128, 128], BF, st6)
            t_s7 = cload([66, 2, 128], BF, st7)
            t_w1 = cload([128, 3, 4, 4, 128], FE4, stw1)
            t_w2 = cload([128, 3, 4, 4, 128], FE4, stw2)

            # ---- per-batch stage emitters; called in a custom order so that
            # batch-0's drain-bound S7 interleaves with batch-1's S1/S2 ----
            st_ = {}

            def alloc_spec(b):
                specB = specp.tile([128, D*KW*C], BF, tag="spec")
                # layout: F = (chi4, cl8, d16, kw33); c = (cph, cpv, cpl, par),
                # chi = (cph, cpv), cl = (cpl, par) -> chi-octet is one
                # single-stride 128-slice for the S3 transposes.
                st_[b, 'specBv'] = specB[:].rearrange(
                    "p (cph cpv cpl par d k) -> p cph cpv cpl par d k",
                    cph=2, cpv=2, cpl=4, par=2, d=D)
                st_[b, 'specBc'] = specB[:].rearrange("p (a k) -> p a k", k=KW)

            def emit_dh_dma(b, dh):
                # rows = (g2, w64) where g = c-half (c = g*16 + cl); HBM runs
                # stay 2KB (c16 x h64 contiguous per (g, w, d)).
                xtb = xtbp.tile([128, 4*(C//2)*H], BF, tag="xtb")
                for g in range(2):
                    nc.scalar.dma_start(
                        out=xtb[g*64:(g+1)*64].rearrange(
                            "p (d c h) -> p d c h", d=4, c=C//2),
                        in_=xin[b, dh*4:(dh+1)*4, :, g*16:(g+1)*16].rearrange(
                            "d w c h -> w d c h"))
                st_[b, 'xtb'] = xtb

            def emit_s1s2_dd(b, d_):
                dd = d_ % 4
                xtb = st_[b, 'xtb']
                specBv = st_[b, 'specBv']
                # S1 data-stationary chunk-PAIR: lhsT [128=(g2,w64), 128 =
                # (c2,h64)], rhs block-diag DFT [128, 132=(g2,kwri66)].
                # psum [(c2,h64), (g2,ri2,kw33)] -> movB (ri2, g2, t8, kw33)
                # so the S2 rhs slices are fully CONTIGUOUS [64, 264].
                movB = wp.tile([128, 16*66], BF, tag="movB")
                movBv = movB[:].rearrange("p (rg t k) -> p rg t k", rg=4, t=8)
                for j0 in range(0, 8, 2):
                    ps = psM.tile([128, 264], F32, tag="m")
                    for jj in range(2):
                        j = j0 + jj
                        nc.tensor.matmul(ps[:, jj*132:(jj+1)*132],
                                         xtb[:, dd*1024 + j*128: dd*1024 + (j+1)*128],
                                         t_s1[:], start=True, stop=True)
                    # psum chunk cols (ri2, g2, kw33); movB cols (ri2, g2, t8, kw33)
                    drain(movBv[:, :, j0:j0+2, :],
                          ps[:].rearrange("p (j rg k) -> p rg j k", j=2, rg=4))
                # S2: par = c parity (lane half), cph = c-half octet.
                # All A-matmuls first, then all B (4 LDW/dd instead of 8);
                # 4 psum banks live (2 from psA + 2 from psB).
                pss = []
                for par in range(2):
                    sl = slice(par*64, (par+1)*64)
                    for cph in range(2):
                        mre = movB[sl, cph*264: (cph+1)*264]
                        mim = movB[sl, 528 + cph*264: 528 + (cph+1)*264]
                        pool = psA if cph == 0 else psB
                        ps = pool.tile([128, 8*KW], F32,
                                       tag="a" if cph == 0 else "b")
                        pss.append((ps, sl, mre, mim))
                for (ps, sl, mre, mim) in pss:
                    nc.tensor.matmul(ps[:], t_s2a[sl, :], mre, start=True, stop=False)
                for (ps, sl, mre, mim) in pss:
                    nc.tensor.matmul(ps[:], t_s2b[sl, :], mim, start=False, stop=True)
                for i, (ps, sl, mre, mim) in enumerate(pss):
                    par, cph = i // 2, i % 2
                    # psum cols (cp8, kw33) -> specB[(chi,cl) c-order]
                    drain(specBv[:, cph, :, :, par, d_, :],
                          ps[:].rearrange("p (v l k) -> p v l k", v=2, l=4))

            def emit_mid(b):
                specBc = st_[b, 'specBc']
                # ============ S2->S3 transpose + S3: fft over D ============
                # y3/y4 are fp8e4: the MLP runs fp8 DoubleRow matmuls (2
                # chi-planes contracted per instruction). Spectral values are
                # O(1) corrections; fp8's ~3% rounding stays well inside the
                # error budget.
                y3 = specp.tile([128, KW*4*2*64], FE4, tag="spec")
                y3v = y3[:].rearrange("p (c r k x) -> p c r k x", c=4, r=2, k=KW)
                y3f = y3[:].rearrange("p (c r kx) -> p c r kx", c=4, r=2)
                for kw_ in range(KW):
                    movT = wp.tile([128, 512], BF, tag="movT")
                    # 4 chi-transposes into one psum bank -> single drain
                    pt = psT.tile([128, 512], BF, tag="t")
                    for chi in range(4):
                        nc.tensor.transpose(
                            pt[:, chi*128:(chi+1)*128],
                            specBc[:, chi*128:(chi+1)*128, kw_], t_id[:])
                    drain(movT[:].rearrange("p (r c x) -> p r c x", r=2, c=4),
                          pt[:].rearrange("p (c r x) -> p r c x", c=4, r=2))
                    mov_re = movT[:, 0:256]
                    mov_im = movT[:, 256:512]
                    pr = psA.tile([128, 256], F32, tag="a")
                    pi = psB.tile([128, 256], F32, tag="b")
                    # s3[0] stationary reused by consecutive matmuls (1 LDW saved)
                    nc.tensor.matmul(pr[:], t_s3[:, 0, :], mov_re, start=True, stop=False)
                    nc.tensor.matmul(pi[:], t_s3[:, 0, :], mov_im, start=True, stop=False)
                    nc.tensor.matmul(pr[:], t_s3[:, 2, :], mov_im, start=False, stop=True)
                    nc.tensor.matmul(pi[:], t_s3[:, 1, :], mov_re, start=False, stop=True)
                    drain(y3v[:, :, 0, kw_, :], pr[:].rearrange("p (c x) -> p c x", c=4))
                    drain(y3v[:, :, 1, kw_, :], pi[:].rearrange("p (c x) -> p c x", c=4))

                # ================= MLP layer 1 (+bias+relu) =================
                y4 = specp.tile([128, KW*4*2*64], FE4, tag="spec")
                y4v = y4[:].rearrange("p (c r k x) -> p c r k x", c=4, r=2, k=KW)
                y4f = y4[:].rearrange("p (c r kx) -> p c r kx", c=4, r=2)
                for q in range(4):
                    for (k0, kn) in KW_CHUNKS:
                        if kn == 1:
                            # psM/psT are idle during the MLP: extra PSUM
                            # rotation depth at the q-pass boundary
                            pr = psM.tile([128, kn*64], F32, tag="m")
                            pi = psT.tile([128, kn*64], F32, tag="t")
                        else:
                            pr = psA.tile([128, kn*64], F32, tag="a")
                            pi = psB.tile([128, kn*64], F32, tag="b")
                        for cp_ in range(2):
                            rre = y3f[:, 2*cp_:2*cp_+2, 0, k0*64:(k0+kn)*64]
                            rim = y3f[:, 2*cp_:2*cp_+2, 1, k0*64:(k0+kn)*64]
                            nc.tensor.matmul(pr[:], t_w1[:, 0, q, 2*cp_:2*cp_+2, :], rre,
                                             start=(cp_ == 0), stop=False, perf_mode=DR)
                            nc.tensor.matmul(pi[:], t_w1[:, 0, q, 2*cp_:2*cp_+2, :], rim,
                                             start=(cp_ == 0), stop=False, perf_mode=DR)
                            nc.tensor.matmul(pr[:], t_w1[:, 2, q, 2*cp_:2*cp_+2, :], rim,
                                             start=False, stop=(cp_ == 1), perf_mode=DR)
                            nc.tensor.matmul(pi[:], t_w1[:, 1, q, 2*cp_:2*cp_+2, :], rre,
                                             start=False, stop=(cp_ == 1), perf_mode=DR)
                        nc.scalar.activation(
                            y4v[:, q, 0, k0:k0+kn, :],
                            pr[:].rearrange("p (k x) -> p k x", k=kn),
                            Relu, bias=t_b1[:, q:q+1], scale=1.0)
                        nc.scalar.activation(
                            y4v[:, q, 1, k0:k0+kn, :],
                            pi[:].rearrange("p (k x) -> p k x", k=kn),
                            Relu, bias=t_b1[:, 4+q:4+q+1], scale=1.0)

                # ================= MLP layer 2 (+bias+softshrink) =================
                y5 = specp.tile([128, KW*4*2*64], BF, tag="spec")
                y5v = y5[:].rearrange("p (c r k x) -> p c r k x", c=4, r=2, k=KW)
                for q in range(4):
                    for (k0, kn) in KW_CHUNKS:
                        if kn == 1:
                            # psM/psT are idle during the MLP: extra PSUM
                            # rotation depth at the q-pass boundary
                            pr = psM.tile([128, kn*64], F32, tag="m")
                            pi = psT.tile([128, kn*64], F32, tag="t")
                        else:
                            pr = psA.tile([128, kn*64], F32, tag="a")
                            pi = psB.tile([128, kn*64], F32, tag="b")
                        for cp_ in range(2):
                            rre = y4f[:, 2*cp_:2*cp_+2, 0, k0*64:(k0+kn)*64]
                            rim = y4f[:, 2*cp_:2*cp_+2, 1, k0*64:(k0+kn)*64]
                            nc.tensor.matmul(pr[:], t_w2[:, 0, q, 2*cp_:2*cp_+2, :], rre,
                                             start=(cp_ == 0), stop=False, perf_mode=DR)
                            nc.tensor.matmul(pi[:], t_w2[:, 0, q, 2*cp_:2*cp_+2, :], rim,
                                             start=(cp_ == 0), stop=False, perf_mode=DR)
                            nc.tensor.matmul(pr[:], t_w2[:, 2, q, 2*cp_:2*cp_+2, :], rim,
                                             start=False, stop=(cp_ == 1), perf_mode=DR)
                            nc.tensor.matmul(pi[:], t_w2[:, 1, q, 2*cp_:2*cp_+2, :], rre,
                                             start=False, stop=(cp_ == 1), perf_mode=DR)
                        for (ps_, ri) in ((pr, 0), (pi, 1)):
                            ta = wp.tile([128, 512], BF, tag="ssA")
                            tb = wp.tile([128, 512], BF, tag="ssB")
                            nc.scalar.activation(ta[:, :kn*64], ps_[:], Relu,
                                                 bias=t_b2m[:, ri*4+q:ri*4+q+1], scale=1.0)
                            nc.scalar.activation(tb[:, :kn*64], ps_[:], Relu,
                                                 bias=t_b2p[:, ri*4+q:ri*4+q+1], scale=-1.0)
                            nc.vector.tensor_sub(
                                y5v[:, q, ri, k0:k0+kn, :],
                                ta[:, :kn*64].rearrange("p (k x) -> p k x", k=kn),
                                tb[:, :kn*64].rearrange("p (k x) -> p k x", k=kn))

                # ================= S5: ifft over D =================
                specE = specp.tile([128, 4*KW*2*64], BF, tag="spec")
                specEv = specE[:].rearrange("p (q k r x) -> p q k r x", q=4, k=KW, r=2)
                for q in range(4):
                    for (k0, kn) in KW_CHUNKS:
                        pr = psA.tile([128, kn*64], F32, tag="a")
                        pi = psB.tile([128, kn*64], F32, tag="b")
                        rre = y5v[:, q, 0, k0:k0+kn, :]
                        rim = y5v[:, q, 1, k0:k0+kn, :]
                        nc.tensor.matmul(pr[:], t_s5[:, 0, :], rre, start=True, stop=False)
                        nc.tensor.matmul(pi[:], t_s5[:, 0, :], rim, start=True, stop=False)
                        nc.tensor.matmul(pr[:], t_s5[:, 2, :], rim, start=False, stop=True)
                        nc.tensor.matmul(pi[:], t_s5[:, 1, :], rre, start=False, stop=True)
                        drain(specEv[:, q, k0:k0+kn, 0, :],
                              pr[:].rearrange("p (k x) -> p k x", k=kn))
                        drain(specEv[:, q, k0:k0+kn, 1, :],
                              pi[:].rearrange("p (k x) -> p k x", k=kn))

                st_[b, 'specE'] = specE

            def emit_s6_q(b, q):
                # ============ S5->S6 transpose + S6: ifft over H (C2) ============
                if q == 0:
                    specF = specp.tile([128, 4*8*8*KW*2], BF, tag="spec")
                    st_[b, 'specF'] = specF
                specE = st_[b, 'specE']
                specF = st_[b, 'specF']
                # Phase 1: all 33 transposes, 4 per psum bank -> batched
                # contiguous drains.
                meb = mebp.tile([128, KW*128], BF, tag="meb")
                for kw0 in range(0, KW, 4):
                    kn = min(4, KW - kw0)
                    pt = psT.tile([128, 512], BF, tag="t")
                    for kk in range(kn):
                        nc.tensor.transpose(
                            pt[:, kk*128:(kk+1)*128],
                            specE[:, (q*KW + kw0 + kk)*128:
                                     (q*KW + kw0 + kk + 1)*128],
                            t_id[:])
                    drain(meb[:, kw0*128:(kw0+kn)*128], pt[:, :kn*128])
                # Phase 2: matmuls share ONE persistent stationary (t_s6
                # stays loaded), 4 kw per psum bank -> batched drains.
                for kw0 in range(0, KW, 4):
                    kn = min(4, KW - kw0)
                    ps6 = psM.tile([128, 512], F32, tag="m")
                    for kk in range(kn):
                        nc.tensor.matmul(ps6[:, kk*128:(kk+1)*128], t_s6[:],
                                         meb[:, (kw0+kk)*128:(kw0+kk+1)*128],
                                         start=True, stop=True)
                    drain(specF[:].rearrange("p (q ce k l) -> p q ce k l",
                                             q=4, ce=64, k=KW)[:, q, :, kw0:kw0+kn, :],
                          ps6[:, :kn*128].rearrange(
                              "p (kk ce l) -> p ce kk l", kk=kn, ce=64))

            def emit_s7_group(b, gi):
                # ============ S6->S7 transpose + S7: irfft over W ============
                if gi == 0:
                    outbuf = outbp.tile([128, 4*8*8*H], BF, tag="outbuf")
                    st_[b, 'outbuf'] = outbuf
                specF = st_[b, 'specF']
                outbuf = st_[b, 'outbuf']
                q, cl = gi // 8, gi % 8
                movF = wp.tile([66, 8*128], BF, tag="movF")
                for e0 in range(0, 8, 4):
                    # 4 transposes per psum bank -> one contiguous drain
                    pt = psT.tile([128, 512], BF, tag="t")
                    for ee in range(4):
                        base = (gi*8 + e0 + ee)*66
                        nc.tensor.transpose(pt[:66, ee*128:(ee+1)*128],
                                            specF[:, base:base+66], t_id[:])
                    drain(movF[:, e0*128:(e0+4)*128], pt[:66, :])
                movFv = movF[:].rearrange("p (e g) -> p e g", e=8)
                ps7 = psM.tile([128, 512], F32, tag="m")
                # alternate stationary order by group parity so adjacent
                # groups share a loaded stationary (halves LDW count)
                if gi % 2 == 0:
                    nc.tensor.matmul(ps7[:], t_s7[:, 0, :], movFv[:, :, 0:64],
                                     start=True, stop=False)
                    nc.tensor.matmul(ps7[:], t_s7[:, 1, :], movFv[:, :, 64:128],
                                     start=False, stop=True)
                else:
                    nc.tensor.matmul(ps7[:], t_s7[:, 1, :], movFv[:, :, 64:128],
                                     start=True, stop=False)
                    nc.tensor.matmul(ps7[:], t_s7[:, 0, :], movFv[:, :, 0:64],
                                     start=False, stop=True)
                # all outbuf drains on one engine so the eps DMA
                # waits on a single producer sem
                drain(outbuf[:, gi*8*H:(gi+1)*8*H], ps7[:], eng=0)
                if cl == 7:
                    # per-q quarter DMA as soon as its 8 cl-drains land
                    epsv = eps[b].rearrange("l w q c e h -> (l w) (q c e h)")
                    nc.sync.dma_start(
                        out=epsv[:, q*8*8*H:(q+1)*8*8*H],
                        in_=outbuf[:, q*8*8*H:(q+1)*8*8*H])

            # ---- emission schedule: batch 0 straight through to S6, then
            # batch-0 S7 groups interleaved with batch-1 S1/S2 (keeps the
            # in-order PE queue fed while S7's drains run on DVE/ACT), then
            # batch-1 mid stages and its S7 tail. ----
            alloc_spec(0)
            for dh in range(4):
                emit_dh_dma(0, dh)
                for dd in range(4):
                    emit_s1s2_dd(0, dh*4 + dd)
            emit_mid(0)
            for q in range(4):
                emit_s6_q(0, q)
            alloc_spec(1)
            for gi in range(32):
                emit_s7_group(0, gi)
                if gi % 2 == 0:
                    d_ = gi // 2
                    if d_ % 4 == 0:
                        emit_dh_dma(1, d_ // 4)
                    emit_s1s2_dd(1, d_)
            emit_mid(1)
            for q in range(4):
                emit_s6_q(1, q)
            for gi in range(32):
                emit_s7_group(1, gi)
    nc.compile()   # bacc passes: splits >1-wait instructions (HW limit)
    return nc


def _get_program():
    if 'nc' not in _PROGRAM_CACHE:
        _PROGRAM_CACHE['nc'] = _build_program()
    return _PROGRAM_CACHE['nc']


# ------------------------------------------------------------------
# host entry point
# ------------------------------------------------------------------

def make_core_inputs(x, w1, b1, w2, b2, n):
    """Build the per-core input map (numpy arrays) for core n."""
    st = _host_stationaries()
    xc = np.asarray(x[..., n*32:(n+1)*32], np.float32)       # [2, D, H, W, 32]
    xt = np.ascontiguousarray(xc.transpose(0, 1, 3, 4, 2))   # [2, D, W, C, H]
    m = {'xin': xt.astype(BF16NP)}
    for k in ('st1', 'st2a', 'st2b', 'st3', 'st5', 'st6', 'st7', 'ident'):
        m[k] = st[k].astype(BF16NP)
    mm = _host_mlp_stationaries(np.asarray(w1), np.asarray(w2), n)
    m['stw1'] = mm['stw1'].astype(ml_dtypes.float8_e4m3)
    m['stw2'] = mm['stw2'].astype(ml_dtypes.float8_e4m3)
    bb = _host_biases(np.asarray(b1), np.asarray(b2), n)
    m['bias1'] = bb['bias1']
    m['bias2m'] = bb['bias2m']
    m['bias2p'] = bb['bias2p']
    m['bias2c'] = bb['bias2c']
    return m


def eps_to_full(eps_n):
    """eps [2, dlo2, w64, q4, cl8, dhi8, h64] -> [2,D,H,W,32] fp32."""
    e = np.asarray(eps_n, dtype=np.float32)
    e = e.transpose(0, 5, 1, 6, 2, 3, 4)      # [b, dhi, dlo, h, w, q, cl]
    return np.ascontiguousarray(e.reshape(2, D, H, W, 32))


def kernel(x, w1, b1, w2, b2):
    from concourse.bass_utils import run_bass_kernel_spmd
    nc = _get_program()
    x = np.asarray(x)
    in_maps = [make_core_inputs(x, w1, b1, w2, b2, n) for n in range(N_CORES)]
    res = run_bass_kernel_spmd(nc, in_maps, list(range(N_CORES)))
    out = np.empty(x.shape, np.float32)
    for n in range(N_CORES):
        eps_full = eps_to_full(res.results[n]['eps'])
        out[..., n*32:(n+1)*32] = x[..., n*32:(n+1)*32].astype(np.float32) + eps_full
    return out

